# revision 1
# baseline (speedup 1.0000x reference)
import numpy as np

TOPK = 8      # n_activated_experts
NCORES = 8    # token-parallel across 8 NeuronCores; weight/bias replicated


def _compute_np(x, weight, bias):
    scores = x @ weight.T + bias
    m = scores.max(axis=-1, keepdims=True)
    e = np.exp(scores - m)
    probs = (e / e.sum(axis=-1, keepdims=True)).astype(np.float32)
    # stable argsort on -probs matches lax.top_k tie-breaking (lowest index first)
    idx = np.argsort(-probs, axis=-1, kind="stable")[:, :TOPK].astype(np.int32)
    return probs, idx


def kernel(x, weight, bias):
    x = np.asarray(x, dtype=np.float32)
    weight = np.asarray(weight, dtype=np.float32)
    bias = np.asarray(bias, dtype=np.float32)
    try:
        import jax
        import jax.numpy as jnp

        def gate(xs, w, b):
            scores = jnp.einsum("td,ed->te", xs, w) + b
            probs = jax.nn.softmax(scores, axis=-1)
            _, indices = jax.lax.top_k(probs, TOPK)
            return probs, indices

        devs = jax.devices()
        tokens = x.shape[0]
        if len(devs) >= NCORES and tokens % NCORES == 0:
            pf = jax.pmap(gate, in_axes=(0, None, None), devices=devs[:NCORES])
            xs = x.reshape(NCORES, tokens // NCORES, x.shape[1])
            probs, idx = pf(xs, weight, bias)
            probs = np.asarray(probs).reshape(tokens, -1).astype(np.float32)
            idx = np.asarray(idx).reshape(tokens, -1).astype(np.int32)
        else:
            probs, idx = jax.jit(gate)(x, weight, bias)
            probs = np.asarray(probs).astype(np.float32)
            idx = np.asarray(idx).astype(np.int32)
        return probs, idx
    except Exception:
        return _compute_np(x, weight, bias)
